# revision 2
# baseline (speedup 1.0000x reference)
"""Causal attention kernel for Trainium2, 8 NeuronCores.

Problem: B=4, H=16, S=2048, D=64 fp32 causal attention.
Sharding: batch*heads (64) split 8 per core; each core computes its 8 heads
independently (no collectives).

Per-core algorithm (per head, S^T layout so softmax reduces over PSUM
partitions via a ones-column appended to V):
  - load Q,K,V; PE-transpose Q,K to [64, 2048] (d on partitions), round to f32r
  - for each 512-wide q block:
      for each 128-tall k tile (causal: k_tile <= q_block end):
        S^T[k,q] = K_tile^T.T @ Q^T          (PE, f32r, N=512)
        P^T = exp(S^T / 8)                   (ACT, PSUM->SBUF, f32r out)
        zero non-causal wedge of P^T         (GPSIMD affine_select)
        O^T[65, q] += [V_tile | 1].T @ P^T   (PE accumulate, f32r)
      evict O^T to SBUF (DVE), PE-transpose back to [q, 65],
      divide by column 64 (the softmax denominator), DMA out.

Heads are processed in pairs sharing ACT instructions ([128, 1024] over two
PSUM banks) to amortize the ~240ns/instr ACT overhead.
"""
import numpy as np
from contextlib import ExitStack

import concourse.bass as bass
import concourse.tile as tile
from concourse import bacc, mybir
from concourse.bass_utils import run_bass_kernel_spmd
from concourse.masks import make_identity

B, H, S, D = 4, 16, 2048, 64
N_CORES = 8
HEADS_PER_CORE = B * H // N_CORES  # 8
P = 128
QB = 512                 # q block width
N_QT = S // P            # 16 s-tiles of 128
N_QB = S // QB           # 4 q blocks
DV = D + 1               # V plus ones column

F32 = mybir.dt.float32
F32R = mybir.dt.float32r

# 'gpsimd' -> zero the non-causal wedge of P^T post-exp via affine_select on a
# bitcast view; 'dve' -> additive -inf wedge masks on the scores pre-exp.
MASK_MODE = "gpsimd"

_cached = None


def build_core_kernel():
    nc = bacc.Bacc("TRN2", target_bir_lowering=False, debug=False)
    q_d = nc.dram_tensor("q", [HEADS_PER_CORE, S, D], F32, kind="ExternalInput")
    k_d = nc.dram_tensor("k", [HEADS_PER_CORE, S, D], F32, kind="ExternalInput")
    v_d = nc.dram_tensor("v", [HEADS_PER_CORE, S, D], F32, kind="ExternalInput")
    o_d = nc.dram_tensor("o", [HEADS_PER_CORE, S, D], F32, kind="ExternalOutput")

    with tile.TileContext(nc) as tc, ExitStack() as ctx:
        const = ctx.enter_context(tc.tile_pool(name="const", bufs=1))
        io = ctx.enter_context(tc.tile_pool(name="io", bufs=4))
        tr = ctx.enter_context(tc.tile_pool(name="tr", bufs=3))
        ptp = ctx.enter_context(tc.tile_pool(name="ptp", bufs=3))
        outp = ctx.enter_context(tc.tile_pool(name="outp", bufs=2))
        st_pool = ctx.enter_context(tc.tile_pool(name="st", bufs=2, space="PSUM"))
        ot_pool = ctx.enter_context(tc.tile_pool(name="ot", bufs=1, space="PSUM"))
        aux_pool = ctx.enter_context(tc.tile_pool(name="aux", bufs=2, space="PSUM"))

        ident = const.tile([P, P], F32)
        make_identity(nc, ident[:])
        if MASK_MODE == "dve":
            # wedge[j][k, q] = 0 where q >= 128*j + k else -1e9, shape [128, 512]
            wedges = []
            for j in range(QB // P):
                w = const.tile([P, QB], F32, tag=f"wedge{j}")
                nc.gpsimd.memset(w[:], 0.0)
                nc.gpsimd.affine_select(
                    out=w[:], in_=w[:],
                    compare_op=mybir.AluOpType.is_ge,
                    fill=-1e9, base=-P * j,
                    pattern=[[1, QB]], channel_multiplier=-1,
                )
                wedges.append(w)

        def load_head(h):
            """DMA one head's Q/K/V, build [64,2048] f32r Q^T/K^T and f32r V'."""
            q_sb = io.tile([P, N_QT, D], F32, tag="q_sb")
            k_sb = io.tile([P, N_QT, D], F32, tag="k_sb")
            v_sb = io.tile([P, N_QT, DV], F32, tag="v_sb")
            nc.sync.dma_start(q_sb[:], q_d[h].rearrange("(t p) d -> p t d", p=P))
            nc.sync.dma_start(k_sb[:], k_d[h].rearrange("(t p) d -> p t d", p=P))
            nc.gpsimd.memset(v_sb[:], 1.0)
            nc.sync.dma_start(
                v_sb[:, :, 0:D], v_d[h].rearrange("(t p) d -> p t d", p=P)
            )
            vr = tr.tile([P, N_QT, DV], F32R, tag="vr")
            nc.vector.tensor_copy(vr[:], v_sb[:])

            qt = tr.tile([D, S], F32R, tag="qt")
            kt = tr.tile([D, S], F32R, tag="kt")
            for src, dst in ((q_sb, qt), (k_sb, kt)):
                for c in range(N_QT // 4):
                    tp = aux_pool.tile([P, QB], F32, tag="aux")
                    for i in range(4):
                        nc.tensor.transpose(
                            tp[0:D, bass.ts(i, P)], src[:, 4 * c + i, :], ident[:]
                        )
                    nc.vector.tensor_copy(dst[:, bass.ts(c, QB)], tp[0:D, :])
            return qt, kt, vr

        def compute_pair(heads_data, pair_heads):
            """Attention for two heads sharing ACT instructions."""
            for qb in range(N_QB):
                ot = ot_pool.tile([DV, 2 * QB], F32)
                nkt = 4 * (qb + 1)
                for kt_i in range(nkt):
                    st = st_pool.tile([P, 2 * QB], F32)
                    for s_i, (qt, kt, vr) in enumerate(heads_data):
                        nc.tensor.matmul(
                            st[:, bass.ts(s_i, QB)],
                            kt[:, bass.ts(kt_i, P)],
                            qt[:, bass.ts(qb, QB)],
                            start=True, stop=True,
                        )
                    j = kt_i - 4 * qb
                    if MASK_MODE == "dve" and j >= 0:
                        for s_i in range(2):
                            nc.vector.tensor_add(
                                st[:, bass.ts(s_i, QB)],
                                st[:, bass.ts(s_i, QB)],
                                wedges[j][:],
                            )
                    pt = ptp.tile([P, 2 * QB], F32R, tag="pt")
                    nc.scalar.activation(
                        pt[:], st[:], mybir.ActivationFunctionType.Exp, scale=0.125
                    )
                    if MASK_MODE == "gpsimd" and j >= 0:
                        w = P * (j + 1)
                        for s_i in range(2):
                            sl = pt[:, s_i * QB : s_i * QB + w]
                            nc.gpsimd.affine_select(
                                out=sl, in_=sl,
                                compare_op=mybir.AluOpType.is_ge,
                                fill=0.0, base=-P * j,
                                pattern=[[1, w]], channel_multiplier=-1,
                            )
                    for s_i, (qt, kt, vr) in enumerate(heads_data):
                        nc.tensor.matmul(
                            ot[:, bass.ts(s_i, QB)],
                            vr[:, kt_i, :],
                            pt[:, bass.ts(s_i, QB)],
                            start=(kt_i == 0), stop=(kt_i == nkt - 1),
                        )
                osb = outp.tile([DV, 2 * QB], F32, tag="osb")
                nc.vector.tensor_copy(osb[:], ot[:])
                for s_i, h in enumerate(pair_heads):
                    res = outp.tile([P, 4, D], F32, tag="res")
                    for i in range(4):
                        otr = aux_pool.tile([P, QB], F32, tag="aux")
                        nc.tensor.transpose(
                            otr[:, 0:DV],
                            osb[:, s_i * QB + i * P : s_i * QB + (i + 1) * P],
                            ident[0:DV, 0:DV],
                        )
                        rec = outp.tile([P, 1], F32, tag="rec")
                        nc.vector.reciprocal(rec[:], otr[:, D : D + 1])
                        nc.vector.tensor_scalar_mul(res[:, i, :], otr[:, 0:D], rec[:])
                    nc.sync.dma_start(
                        o_d[h].rearrange("(t p) d -> p t d", p=P)[:, 4 * qb : 4 * qb + 4, :],
                        res[:],
                    )

        for pair in range(HEADS_PER_CORE // 2):
            hs = (2 * pair, 2 * pair + 1)
            data = [load_head(h) for h in hs]
            compute_pair(data, hs)

    nc.compile()
    return nc


def kernel(q, k, v):
    global _cached
    q = np.asarray(q, dtype=np.float32).reshape(B * H, S, D)
    k = np.asarray(k, dtype=np.float32).reshape(B * H, S, D)
    v = np.asarray(v, dtype=np.float32).reshape(B * H, S, D)

    if _cached is None:
        _cached = build_core_kernel()
    nc = _cached

    in_maps = []
    for c in range(N_CORES):
        sl = slice(c * HEADS_PER_CORE, (c + 1) * HEADS_PER_CORE)
        in_maps.append({
            "q": np.ascontiguousarray(q[sl]),
            "k": np.ascontiguousarray(k[sl]),
            "v": np.ascontiguousarray(v[sl]),
        })
    res = run_bass_kernel_spmd(nc, in_maps, core_ids=list(range(N_CORES)))
    out = np.concatenate([r["o"] for r in res.results], axis=0)
    return out.reshape(B, H, S, D)


# revision 3
# speedup vs baseline: 23.4026x; 23.4026x over previous
"""Causal attention kernel for Trainium2, 8 NeuronCores.

Problem: B=4, H=16, S=2048, D=64 fp32 causal attention.
Sharding: batch*heads (64) split 8 per core; each core computes its 8 heads
independently (no collectives).

Per-core algorithm (per head pair, S^T layout so softmax reduces over PSUM
partitions via a ones-column appended to V):
  - load Q,K,V for two heads; PE-transpose into combined [128, 2048] tiles
    (head A on partitions 0:64, head B on 64:128), rounded to f32r
  - for each 512-wide q block, for each 128-tall k tile (causal):
      S^T[k,q] = K_tile^T.T @ Q^T     two row-packed f32r matmuls (concurrent
                                      in the PE array: row groups 0:64 / 64:128)
      P^T = exp(S^T / 8)              one ACT instr covering both heads
      zero triangle of diagonal tiles (GPSIMD affine_select, f32r)
      O^T[65, q] += [V_tile | 1].T @ P^T   (PE accumulate; row 64 = denom)
    evict O^T to SBUF (DVE), PE-transpose back to [q, 65], divide by the
    denominator column, DMA out.
"""
import numpy as np
from contextlib import ExitStack

import concourse.bass as bass
import concourse.tile as tile
from concourse import bacc, mybir
from concourse.bass_utils import run_bass_kernel_spmd
from concourse.masks import make_identity

B, H, S, D = 4, 16, 2048, 64
N_CORES = 8
HEADS_PER_CORE = B * H // N_CORES  # 8
P = 128
QB = 512                 # q block width
N_QT = S // P            # 16 s-tiles of 128
N_QB = S // QB           # 4 q blocks
DV = D + 1               # V plus ones column

F32 = mybir.dt.float32
F32R = mybir.dt.float32r

_cached = None


def build_core_kernel():
    nc = bacc.Bacc("TRN2", target_bir_lowering=False, debug=False)
    q_d = nc.dram_tensor("q", [HEADS_PER_CORE, S, D], F32, kind="ExternalInput")
    k_d = nc.dram_tensor("k", [HEADS_PER_CORE, S, D], F32, kind="ExternalInput")
    v_d = nc.dram_tensor("v", [HEADS_PER_CORE, S, D], F32, kind="ExternalInput")
    o_d = nc.dram_tensor("o", [HEADS_PER_CORE, S, D], F32, kind="ExternalOutput")

    with tile.TileContext(nc) as tc, ExitStack() as ctx:
        const = ctx.enter_context(tc.tile_pool(name="const", bufs=1))
        io = ctx.enter_context(tc.tile_pool(name="io", bufs=4))
        tr = ctx.enter_context(tc.tile_pool(name="tr", bufs=2))
        ptp = ctx.enter_context(tc.tile_pool(name="ptp", bufs=4))
        outp = ctx.enter_context(tc.tile_pool(name="outp", bufs=2))
        st_pool = ctx.enter_context(tc.tile_pool(name="st", bufs=2, space="PSUM"))
        ot_pool = ctx.enter_context(tc.tile_pool(name="ot", bufs=1, space="PSUM"))
        aux_pool = ctx.enter_context(tc.tile_pool(name="aux", bufs=2, space="PSUM"))

        ident = const.tile([P, P], F32)
        make_identity(nc, ident[:])

        def load_pair(hA, hB):
            """DMA two heads' Q/K/V; build combined f32r Q^T/K^T [128, 2048]
            (head A on partitions 0:64, head B on 64:128) and per-head V'."""
            vrs = []
            srcs = {}
            for s_i, h in enumerate((hA, hB)):
                q_sb = io.tile([P, N_QT, D], F32, tag=f"q_sb{s_i}")
                k_sb = io.tile([P, N_QT, D], F32, tag=f"k_sb{s_i}")
                v_sb = io.tile([P, N_QT, DV], F32, tag=f"v_sb{s_i}")
                nc.sync.dma_start(q_sb[:], q_d[h].rearrange("(t p) d -> p t d", p=P))
                nc.sync.dma_start(k_sb[:], k_d[h].rearrange("(t p) d -> p t d", p=P))
                nc.gpsimd.memset(v_sb[:], 1.0)
                nc.sync.dma_start(
                    v_sb[:, :, 0:D], v_d[h].rearrange("(t p) d -> p t d", p=P)
                )
                vr = tr.tile([P, N_QT, DV], F32R, tag=f"vr{s_i}")
                nc.vector.tensor_copy(vr[:], v_sb[:])
                vrs.append(vr)
                srcs[s_i] = (q_sb, k_sb)

            qt = tr.tile([P, S], F32R, tag="qt")
            kt = tr.tile([P, S], F32R, tag="kt")
            for which, dst in ((0, qt), (1, kt)):
                for c in range(N_QT // 4):
                    tp = aux_pool.tile([P, QB], F32, tag="aux")
                    for i in range(4):
                        t_idx = 4 * c + i
                        nc.tensor.transpose(
                            tp[0:D, bass.ts(i, P)],
                            srcs[0][which][:, t_idx, :],
                            ident[:],
                        )
                        nc.tensor.transpose(
                            tp[D:P, bass.ts(i, P)],
                            srcs[1][which][:, t_idx, :],
                            ident[:],
                            tile_position=(0, D),
                        )
                    nc.vector.tensor_copy(dst[:, bass.ts(c, QB)], tp[:])
            return qt, kt, vrs

        def compute_pair(qt, kt, vrs, pair_heads):
            for qb in range(N_QB):
                ot = ot_pool.tile([DV, 2 * QB], F32)
                nkt = 4 * (qb + 1)
                for kt_i in range(nkt):
                    j = kt_i - 4 * qb  # >= 0 on diagonal tiles
                    off = P * j if j > 0 else 0
                    w = QB - off
                    st = st_pool.tile([P, 2, QB], F32)
                    for s_i in range(2):
                        lo, hi = s_i * D, s_i * D + D
                        nc.tensor.matmul(
                            st[:, s_i, off:QB],
                            kt[lo:hi, bass.ts(kt_i, P)],
                            qt[lo:hi, QB * qb + off : QB * qb + QB],
                            start=True, stop=True,
                        )
                    pt = ptp.tile([P, 2, QB], F32R, tag="pt")
                    nc.scalar.activation(
                        pt[:, :, off:QB], st[:, :, off:QB],
                        mybir.ActivationFunctionType.Exp, scale=0.125,
                    )
                    if j >= 0:
                        for s_i in range(2):
                            sl = pt[:, s_i, off : off + P]
                            nc.gpsimd.affine_select(
                                out=sl, in_=sl,
                                compare_op=mybir.AluOpType.is_ge,
                                fill=0.0, base=0,
                                pattern=[[1, P]], channel_multiplier=-1,
                            )
                    for s_i in range(2):
                        nc.tensor.matmul(
                            ot[:, QB * s_i + off : QB * s_i + QB],
                            vrs[s_i][:, kt_i, :],
                            pt[:, s_i, off:QB],
                            start=(kt_i == 0), stop=(kt_i == nkt - 1),
                        )
                osb = outp.tile([DV, 2 * QB], F32, tag="osb")
                nc.vector.tensor_copy(osb[:], ot[:])
                for s_i, h in enumerate(pair_heads):
                    res = outp.tile([P, 4, D], F32, tag="res")
                    for i in range(4):
                        otr = aux_pool.tile([P, QB], F32, tag="aux")
                        nc.tensor.transpose(
                            otr[:, 0:DV],
                            osb[:, s_i * QB + i * P : s_i * QB + (i + 1) * P],
                            ident[0:DV, 0:DV],
                        )
                        rec = outp.tile([P, 1], F32, tag="rec")
                        nc.vector.reciprocal(rec[:], otr[:, D : D + 1])
                        nc.vector.tensor_scalar_mul(res[:, i, :], otr[:, 0:D], rec[:])
                    nc.sync.dma_start(
                        o_d[h].rearrange("(t p) d -> p t d", p=P)[:, 4 * qb : 4 * qb + 4, :],
                        res[:],
                    )

        for pair in range(HEADS_PER_CORE // 2):
            hA, hB = 2 * pair, 2 * pair + 1
            qt, kt, vrs = load_pair(hA, hB)
            compute_pair(qt, kt, vrs, (hA, hB))

    nc.compile()
    return nc


def kernel(q, k, v):
    global _cached
    q = np.asarray(q, dtype=np.float32).reshape(B * H, S, D)
    k = np.asarray(k, dtype=np.float32).reshape(B * H, S, D)
    v = np.asarray(v, dtype=np.float32).reshape(B * H, S, D)

    if _cached is None:
        _cached = build_core_kernel()
    nc = _cached

    in_maps = []
    for c in range(N_CORES):
        sl = slice(c * HEADS_PER_CORE, (c + 1) * HEADS_PER_CORE)
        in_maps.append({
            "q": np.ascontiguousarray(q[sl]),
            "k": np.ascontiguousarray(k[sl]),
            "v": np.ascontiguousarray(v[sl]),
        })
    res = run_bass_kernel_spmd(nc, in_maps, core_ids=list(range(N_CORES)))
    out = np.concatenate([r["o"] for r in res.results], axis=0)
    return out.reshape(B, H, S, D)


# revision 5
# speedup vs baseline: 4786.9503x; 204.5479x over previous
"""Causal attention kernel for Trainium2, 8 NeuronCores.

Problem: B=4, H=16, S=2048, D=64 fp32 causal attention.
Sharding: batch*heads (64) split 8 per core; each core computes its 8 heads
independently (no collectives).

Per-core algorithm (heads processed in pairs, S^T layout so softmax reduces
over PSUM partitions via a ones-column appended to V):
  - load Q,K for two heads into packed [128s, 128d] tiles; one regular matmul
    against identity transposes BOTH heads at once (head A -> partitions 0:64,
    head B -> 64:128); evict to f32r Q^T/K^T [128, 2048]
  - for each 512-wide q block, for each 128-tall k tile (causal):
      S^T[k,q] = K_tile^T.T @ Q^T     two row-packed f32r matmuls (concurrent
                                      in the PE array: row groups 0:64 / 64:128)
      P^T = exp(S^T / 8)              one ACT instr covering both heads
      zero triangle of diagonal tiles (GPSIMD affine_select, f32r)
      O^T[65, q] += [V_tile | 1].T @ P^T   (PE accumulate; row 64 = denom)
    evict O^T to SBUF (DVE), PE-transpose back to [q, 65], divide by the
    denominator column, DMA out.
"""
import numpy as np
from contextlib import ExitStack

import concourse.bass as bass
import concourse.tile as tile
from concourse import bacc, mybir
from concourse.bass_utils import run_bass_kernel_spmd
from concourse.masks import make_identity

B, H, S, D = 4, 16, 2048, 64
N_CORES = 8
HEADS_PER_CORE = B * H // N_CORES  # 8
P = 128
QB = 512                 # q block width
N_QT = S // P            # 16 s-tiles of 128
N_QB = S // QB           # 4 q blocks
N_CH = N_QT // 4         # 4 s-tile chunks per head
DV = D + 1               # V plus ones column

F32 = mybir.dt.float32
F32R = mybir.dt.float32r

_cached = None


def build_core_kernel(repeat_n=None):
    nc = bacc.Bacc("TRN2", target_bir_lowering=False, debug=False)
    q_d = nc.dram_tensor("q", [HEADS_PER_CORE, S, D], F32, kind="ExternalInput")
    k_d = nc.dram_tensor("k", [HEADS_PER_CORE, S, D], F32, kind="ExternalInput")
    v_d = nc.dram_tensor("v", [HEADS_PER_CORE, S, D], F32, kind="ExternalInput")
    o_d = nc.dram_tensor("o", [HEADS_PER_CORE, S, D], F32, kind="ExternalOutput")

    with tile.TileContext(nc) as tc, ExitStack() as ctx:
        const = ctx.enter_context(tc.tile_pool(name="const", bufs=1))
        ioqk = ctx.enter_context(tc.tile_pool(name="ioqk", bufs=6))
        iov = ctx.enter_context(tc.tile_pool(name="iov", bufs=2))
        tr = ctx.enter_context(tc.tile_pool(name="tr", bufs=2))
        ptp = ctx.enter_context(tc.tile_pool(name="ptp", bufs=4))
        outp = ctx.enter_context(tc.tile_pool(name="outp", bufs=2))
        st_pool = ctx.enter_context(tc.tile_pool(name="st", bufs=2, space="PSUM"))
        ot_pool = ctx.enter_context(tc.tile_pool(name="ot", bufs=1, space="PSUM"))
        tin_pool = ctx.enter_context(tc.tile_pool(name="tin", bufs=1, space="PSUM"))
        tout_pool = ctx.enter_context(tc.tile_pool(name="tout", bufs=1, space="PSUM"))

        ident = const.tile([P, P], F32)
        make_identity(nc, ident[:])

        def load_pair(hA, hB):
            """DMA two heads' Q/K/V; build combined f32r Q^T/K^T [128, 2048]
            (head A on partitions 0:64, head B on 64:128) and per-head V'.

            Packed transpose: one REGULAR matmul per s-tile:
            lhsT = [Q_A_tile | Q_B_tile] [128s, 128d], rhs = identity ->
            out[m, s] = lhsT[s, m]: head A rows 0:64, head B rows 64:128.
            """
            vrs = []
            for s_i, h in enumerate((hA, hB)):
                v_sb = iov.tile([P, N_QT, DV], F32, tag=f"v_sb{s_i}")
                nc.gpsimd.memset(v_sb[:], 1.0)
                nc.sync.dma_start(
                    v_sb[:, :, 0:D], v_d[h].rearrange("(t p) d -> p t d", p=P)
                )
                vr = tr.tile([P, N_QT, DV], F32R, tag=f"vr{s_i}")
                nc.vector.tensor_copy(vr[:], v_sb[:])
                vrs.append(vr)

            qt = tr.tile([P, S], F32R, tag="qt")
            kt = tr.tile([P, S], F32R, tag="kt")
            for src_d, dst in ((q_d, qt), (k_d, kt)):
                for c in range(N_CH):
                    ab = ioqk.tile([P, 4, P], F32, tag="ab")
                    for s_i, h in enumerate((hA, hB)):
                        lo, hi = s_i * D, s_i * D + D
                        nc.sync.dma_start(
                            ab[:, :, lo:hi],
                            src_d[h].rearrange("(t p) d -> p t d", p=P)[:, 4 * c : 4 * c + 4, :],
                        )
                    tp = tin_pool.tile([P, QB], F32, tag="tin")
                    for i in range(4):
                        nc.tensor.matmul(
                            tp[:, bass.ts(i, P)],
                            ab[:, i, :],
                            ident[:],
                            start=True, stop=True,
                        )
                    nc.vector.tensor_copy(dst[:, bass.ts(c, QB)], tp[:])
            return qt, kt, vrs

        def compute_pair(qt, kt, vrs, pair_heads):
            for qb in range(N_QB):
                ot = ot_pool.tile([DV, 2 * QB], F32)
                nkt = 4 * (qb + 1)
                for kt_i in range(nkt):
                    j = kt_i - 4 * qb  # >= 0 on diagonal tiles
                    off = P * j if j > 0 else 0
                    w = QB - off
                    st = st_pool.tile([P, 2, QB], F32)
                    for s_i in range(2):
                        lo, hi = s_i * D, s_i * D + D
                        nc.tensor.matmul(
                            st[:, s_i, off:QB],
                            kt[lo:hi, bass.ts(kt_i, P)],
                            qt[lo:hi, QB * qb + off : QB * qb + QB],
                            start=True, stop=True,
                        )
                    pt = ptp.tile([P, 2, QB], F32R, tag="pt")
                    nc.scalar.activation(
                        pt[:, :, off:QB], st[:, :, off:QB],
                        mybir.ActivationFunctionType.Exp, scale=0.125,
                    )
                    if j >= 0:
                        for s_i in range(2):
                            sl = pt[:, s_i, off : off + P]
                            nc.gpsimd.affine_select(
                                out=sl, in_=sl,
                                compare_op=mybir.AluOpType.is_ge,
                                fill=0.0, base=0,
                                pattern=[[1, P]], channel_multiplier=-1,
                            )
                    for s_i in range(2):
                        nc.tensor.matmul(
                            ot[:, QB * s_i + off : QB * s_i + QB],
                            vrs[s_i][:, kt_i, :],
                            pt[:, s_i, off:QB],
                            start=(kt_i == 0), stop=(kt_i == nkt - 1),
                        )
                osb = outp.tile([DV, 2 * QB], F32, tag="osb")
                nc.vector.tensor_copy(osb[:], ot[:])
                for s_i, h in enumerate(pair_heads):
                    res = outp.tile([P, 4, D], F32, tag="res")
                    for i in range(4):
                        otr = tout_pool.tile([P, 4 * DV], F32, tag="tout")
                        nc.tensor.transpose(
                            otr[:, 0:DV],
                            osb[:, s_i * QB + i * P : s_i * QB + (i + 1) * P],
                            ident[0:DV, 0:DV],
                        )
                        rec = outp.tile([P, 1], F32, tag="rec")
                        nc.vector.reciprocal(rec[:], otr[:, D : D + 1])
                        nc.vector.tensor_scalar_mul(res[:, i, :], otr[:, 0:D], rec[:])
                    nc.sync.dma_start(
                        o_d[h].rearrange("(t p) d -> p t d", p=P)[:, 4 * qb : 4 * qb + 4, :],
                        res[:],
                    )

        def body():
            for pair in range(HEADS_PER_CORE // 2):
                hA, hB = 2 * pair, 2 * pair + 1
                qt, kt, vrs = load_pair(hA, hB)
                compute_pair(qt, kt, vrs, (hA, hB))

        if repeat_n is None:
            body()
        else:
            with tc.For_i(0, repeat_n, 1):
                body()

    nc.compile()
    return nc


def kernel(q, k, v):
    global _cached
    q = np.asarray(q, dtype=np.float32).reshape(B * H, S, D)
    k = np.asarray(k, dtype=np.float32).reshape(B * H, S, D)
    v = np.asarray(v, dtype=np.float32).reshape(B * H, S, D)

    if _cached is None:
        _cached = build_core_kernel()
    nc = _cached

    in_maps = []
    for c in range(N_CORES):
        sl = slice(c * HEADS_PER_CORE, (c + 1) * HEADS_PER_CORE)
        in_maps.append({
            "q": np.ascontiguousarray(q[sl]),
            "k": np.ascontiguousarray(k[sl]),
            "v": np.ascontiguousarray(v[sl]),
        })
    res = run_bass_kernel_spmd(nc, in_maps, core_ids=list(range(N_CORES)))
    out = np.concatenate([r["o"] for r in res.results], axis=0)
    return out.reshape(B, H, S, D)
